# revision 15
# baseline (speedup 1.0000x reference)
"""Trainium2 Bass kernel: 16-head RoPE attention block (B=4, T=2048, D=2048).

Sharding: tensor-parallel over heads. Each of the 8 cores owns 2 heads
(a 256-wide slice of the q/k/v projection output features) and computes a
full-width partial of the output projection; the host sums the 8 fp16
partials (the "all-reduce").

v3 structure (vs the riffled v2):
  - cross-block score pipeline: all score matmuls + EXPs flow through one
    global rate-limited queue, popped between every ~3 PE matmuls anywhere
    in the schedule. The scalar engine's EXP backlog builds during the
    projection/out-proj phases, so PV matmuls consume pre-built E tiles
    and never starve on the activation engine.
  - softmax denominators from a depth-3 bf16 add tree (8 pairs -> 4 quads
    -> 2 octs on the DVE) + two ones-matmuls per block (was 5).
  - V produced token-major directly by swapping matmul operands
    (stationary = x slice, moving = Wv) - no XBAR DMA transpose.
  - startup: ones via memset (no DRAM const), x chunk 0 / wk / wv split
    into pieces across the 4 DMA rings, ~shorter PE warm-up.
  - tail: the final out-projection block's DMA drains in 4 pieces across
    all 4 rings.
  - everything flows in bf16 (weights, x, q/k/v, exp(S), attention, Wo);
    PSUM accumulation stays fp32; outputs written as fp16 partials.
"""

import math

import numpy as np
import ml_dtypes

import concourse.bacc as bacc
import concourse.bass as bass
import concourse.mybir as mybir
import concourse.tile as tile
from concourse.bass_utils import run_bass_kernel_spmd

F32 = mybir.dt.float32
BF16 = mybir.dt.bfloat16
FP16 = mybir.dt.float16
EXP = mybir.ActivationFunctionType.Exp

# Problem shape (hardcoded; the harness calls kernel() with exactly these).
B = 4
T = 2048
D_MODEL = 2048
HEAD_DIM = 128
N_CORES = 8
ROPE_BASE = 10000.0

HPC = 2                      # heads per core
F_LOC = HPC * HEAD_DIM       # 256 local projection features per core
TCH = 512                    # stage-1 token chunk width
QCH = 512                    # stage-2 query chunk width
SCALE = 1.0 / math.sqrt(HEAD_DIM)

POP_SLOTS = 2.8              # PE 512-col-slots between score pops
E_MAX = 11                   # outstanding (popped, not PV-consumed) E tiles
WARMUP = 150                 # PE warm-up matmuls (p-state ramp)


def build_module(b=B, t=T, d_model=D_MODEL, n_cores=N_CORES):
    """Build the per-core Bass module. All cores run the same program on
    different data (pure SPMD, no collectives)."""
    dt_ = d_model // 128     # 16 contraction tiles
    kt = t // 128            # 16 key tiles per batch
    cpb = t // TCH           # 4 stage-1 chunks per batch
    nqc = t // QCH           # 4 query chunks

    nc = bacc.Bacc(None, target_bir_lowering=False)

    xT = nc.dram_tensor("xT", [d_model, b * t], BF16, kind="ExternalInput")
    wqT = nc.dram_tensor("wqT", [d_model, F_LOC], BF16, kind="ExternalInput")
    wkT = nc.dram_tensor("wkT", [d_model, F_LOC], BF16, kind="ExternalInput")
    wvT = nc.dram_tensor("wvT", [d_model, F_LOC], BF16, kind="ExternalInput")
    woT = nc.dram_tensor("woT", [F_LOC, d_model], BF16, kind="ExternalInput")
    cosT = nc.dram_tensor("cosT", [HEAD_DIM, t], BF16, kind="ExternalInput")
    rsinT = nc.dram_tensor("rsinT", [HEAD_DIM, t], BF16, kind="ExternalInput")
    outP = nc.dram_tensor("outP", [d_model, b * t], FP16, kind="ExternalOutput")

    with tile.TileContext(nc) as tc:
        with (
            tc.tile_pool(name="const", bufs=1) as constp,
            tc.tile_pool(name="wq", bufs=1) as wpool,
            tc.tile_pool(name="x", bufs=2) as xpool,
            tc.tile_pool(name="qkv", bufs=2) as qkvp,
            tc.tile_pool(name="t1", bufs=2) as tpool,
            tc.tile_pool(name="e", bufs=13) as epool,
            tc.tile_pool(name="tr", bufs=10) as trpool,
            tc.tile_pool(name="s2", bufs=2) as s2pool,
            tc.tile_pool(name="attn", bufs=2) as attnp,
            tc.tile_pool(name="s3o", bufs=6) as s3pool,
            tc.tile_pool(name="ps_a", bufs=2, space="PSUM") as ps_a,
            tc.tile_pool(name="ps_sc", bufs=2, space="PSUM") as ps_sc,
            tc.tile_pool(name="ps_pv", bufs=2, space="PSUM") as ps_pv,
            tc.tile_pool(name="ps_dn", bufs=2, space="PSUM") as ps_dn,
        ):
            # ---- constants: ones from memset (no DRAM), so the PE
            # warm-up starts immediately ----
            ones_sb = constp.tile([128, 128], BF16)
            nc.vector.memset(ones_sb, 1.0)

            # PE warm-up: ramp the p-state while the weight/x DMAs land
            warm_ps = ps_dn.tile([128, QCH], F32, tag="dn")
            for wu in range(WARMUP):
                nc.tensor.matmul(
                    warm_ps[:, 0:128], ones_sb, ones_sb, start=True, stop=True
                )

            # ---- initial loads: wk leads sync+scalar, x chunk 0 3-way
            # across the rings (sync/scalar HWDGE + gpsimd SWDGE), wq
            # behind x, wv on gpsimd. Steady-state x rides gpsimd only so
            # the scalar ring never blocks EXP dispatch. ----
            x_first = xpool.tile([128, dt_, TCH], BF16, name="x0_0", tag="x")
            x0src = xT[:, 0:TCH].rearrange("(dt p) tt -> p dt tt", p=128)

            w_sbs = []
            for wten, wname in ((wqT, "wq"), (wkT, "wk"), (wvT, "wv")):
                wsb = wpool.tile([128, dt_, F_LOC], BF16, name=wname, tag=wname)
                w_sbs.append(wsb)
            wsrc = [
                w.rearrange("(dt p) f -> p dt f", p=128)
                for w in (wqT[:, :], wkT[:, :], wvT[:, :])
            ]
            # wk halves first (needed first)
            nc.sync.dma_start(out=w_sbs[1][:, 0:8, :], in_=wsrc[1][:, 0:8, :])
            nc.scalar.dma_start(out=w_sbs[1][:, 8:16, :], in_=wsrc[1][:, 8:16, :])
            # x chunk 0 split 3 ways
            nc.sync.dma_start(out=x_first[:, 0:5, :], in_=x0src[:, 0:5, :])
            nc.scalar.dma_start(out=x_first[:, 5:10, :], in_=x0src[:, 5:10, :])
            nc.gpsimd.dma_start(out=x_first[:, 10:16, :], in_=x0src[:, 10:16, :])
            # wq halves (q projection is second), wv whole on gpsimd
            nc.sync.dma_start(out=w_sbs[0][:, 0:8, :], in_=wsrc[0][:, 0:8, :])
            nc.scalar.dma_start(out=w_sbs[0][:, 8:16, :], in_=wsrc[0][:, 8:16, :])
            nc.gpsimd.dma_start(out=w_sbs[2], in_=wsrc[2])
            # rope tables (bf16) on scalar; wo trails on gpsimd (issued
            # after the x(0,2) load below)
            cos_sb = constp.tile([128, t], BF16)
            nc.scalar.dma_start(out=cos_sb, in_=cosT[:, :])
            rsin_sb = constp.tile([128, t], BF16)
            nc.scalar.dma_start(out=rsin_sb, in_=rsinT[:, :])
            wo_sb = wpool.tile([128, HPC, d_model], BF16, tag="wo")

            # per-batch double-buffered SBUF state, created lazily
            qk_sb = {}       # bi -> (q_sb, k_sb)  [128, HPC, t] bf16
            v_sb = {}        # bi -> [128, kt, HPC, 128] bf16
            attn_sb = {}     # bi -> [128, HPC, t] bf16

            # ============== global score pipeline ======================
            # Every (bi, h, qc, kti) score matmul + EXP flows through this
            # queue in block order. Pops are rate-limited to one per
            # ~POP_SLOTS 512-col PE slots (the EXP drain rate) and gated
            # on (a) the producing s1 chunks being emitted and (b) at most
            # E_MAX un-consumed E tiles outstanding.
            squeue = []
            for bi_ in range(b):
                for h_ in range(HPC):
                    for qc_ in range(nqc):
                        for kti_ in range(kt):
                            squeue.append((bi_, h_, qc_, kti_))
            state = {"head": 0, "slots": 0.0, "e_out": 0}
            chunk_done = set()
            e_reg = {}       # (bi,h,qc) -> {kti: e_tile}
            e_cnt = {}       # (bi,h,qc) -> popped count
            tree = {}        # (bi,h,qc) -> dict(pr=[], qd=[], oct=[])
            dn_ps = {}       # (bi,h,qc) -> dn psum tile

            def tree_update(blk, kti):
                st = tree.setdefault(blk, {"pr": [], "qd": [], "oct": []})
                reg = e_reg[blk]
                if kti % 2 == 1:
                    # pair adds on the otherwise-idle gpsimd engine;
                    # quad/oct adds on the DVE
                    pr = trpool.tile([128, QCH], BF16, tag="tr", name="pr")
                    nc.gpsimd.tensor_add(pr, reg[kti - 1], reg[kti])
                    st["pr"].append(pr)
                    np_ = len(st["pr"])
                    if np_ % 2 == 0:
                        qd = trpool.tile([128, QCH], BF16, tag="tr", name="qd")
                        nc.vector.tensor_add(qd, st["pr"][-2], st["pr"][-1])
                        st["qd"].append(qd)
                        nq_ = len(st["qd"])
                        if nq_ % 2 == 0:
                            oc = trpool.tile([128, QCH], BF16, tag="tr", name="oc")
                            nc.vector.tensor_add(oc, st["qd"][-2], st["qd"][-1])
                            st["oct"].append(oc)

            def emit_dn(blk):
                """Two ones-matmuls over the oct tiles; deferred until the
                PE is safely past the tree's cross-engine latency."""
                st = tree.pop(blk)
                dnp = ps_dn.tile([128, QCH], F32, tag="dn")
                nc.tensor.matmul(dnp, ones_sb, st["oct"][0], start=True, stop=False)
                nc.tensor.matmul(dnp, ones_sb, st["oct"][1], start=False, stop=True)
                dn_ps[blk] = dnp

            def poppable():
                if state["head"] >= len(squeue):
                    return False
                bi_, h_, qc_, kti_ = squeue[state["head"]]
                if (bi_, max(qc_, kti_ // 4)) not in chunk_done:
                    return False
                return state["e_out"] < E_MAX

            def pop_one(force=False):
                if state["head"] >= len(squeue):
                    return False
                bi_, h_, qc_, kti_ = squeue[state["head"]]
                if (bi_, max(qc_, kti_ // 4)) not in chunk_done:
                    return False
                if not force and state["e_out"] >= E_MAX:
                    return False
                state["head"] += 1
                blk = (bi_, h_, qc_)
                q_t, k_t = qk_sb[bi_]
                sps = ps_sc.tile([128, QCH], F32, tag="sc")
                nc.tensor.matmul(
                    sps,
                    k_t[:, h_, kti_ * 128 : (kti_ + 1) * 128],
                    q_t[:, h_, qc_ * QCH : (qc_ + 1) * QCH],
                    start=True,
                    stop=True,
                )
                e_sb = epool.tile([128, QCH], BF16, tag="E", name="e")
                nc.scalar.activation(e_sb, sps, EXP, scale=SCALE)
                e_reg.setdefault(blk, {})[kti_] = e_sb
                e_cnt[blk] = e_cnt.get(blk, 0) + 1
                state["e_out"] += 1
                tree_update(blk, kti_)
                return True

            def maybe_pop(w):
                state["slots"] += w
                while state["slots"] >= POP_SLOTS and pop_one():
                    state["slots"] -= POP_SLOTS
                # when gated or drained, don't bank more than one pop
                state["slots"] = min(state["slots"], POP_SLOTS)

            # ================= emission units =========================
            x_tiles = {}

            def s1_load(bi, c):
                """Issue the x-chunk DMA (placed ahead of its compute).
                Steady state rides the gpsimd SWDGE ring only; chunk (0,1)
                splits sync+gpsimd to beat the startup crunch."""
                off = c * TCH
                tsl = slice(bi * t + off, bi * t + off + TCH)
                x_sb = xpool.tile([128, dt_, TCH], BF16, name=f"x{bi}_{c}", tag="x")
                xsrc = xT[:, tsl].rearrange("(dt p) tt -> p dt tt", p=128)
                if (bi, c) == (0, 1):
                    nc.sync.dma_start(out=x_sb[:, 0:8, :], in_=xsrc[:, 0:8, :])
                    nc.gpsimd.dma_start(out=x_sb[:, 8:16, :], in_=xsrc[:, 8:16, :])
                else:
                    nc.gpsimd.dma_start(out=x_sb, in_=xsrc)
                if (bi, c) == (0, 2):
                    # wo behind the first steady x chunk on gpsimd
                    nc.gpsimd.dma_start(
                        out=wo_sb,
                        in_=woT[:, :].rearrange("(ft p) d -> p ft d", p=128),
                    )
                x_tiles[(bi, c)] = x_sb

            def s1_chunk(bi, c):
                """Projections + rope + token-major V for 512 tokens."""
                if c == 0:
                    qk_sb[bi] = (
                        qkvp.tile([128, HPC, t], BF16, name=f"q{bi}", tag="q"),
                        qkvp.tile([128, HPC, t], BF16, name=f"k{bi}", tag="k"),
                    )
                    v_sb[bi] = qkvp.tile(
                        [128, kt, F_LOC], BF16, name=f"v{bi}", tag="v"
                    )
                off = c * TCH
                lsl = slice(off, off + TCH)
                x_sb = x_tiles.pop((bi, c))

                def proj_rope(pi):
                    for ft in range(HPC):
                        fsl = slice(ft * 128, (ft + 1) * 128)
                        ps = ps_a.tile([128, TCH], F32, tag="a", name="psqk")
                        for di in range(dt_):
                            nc.tensor.matmul(
                                ps,
                                w_sbs[pi][:, di, fsl],
                                x_sb[:, di, :],
                                start=(di == 0),
                                stop=(di == dt_ - 1),
                            )
                            maybe_pop(1.0)
                        # rope: out = in*cos + rot_half(in)*sin
                        ro = tpool.tile([128, TCH], F32, tag="ro")
                        nc.vector.tensor_mul(ro, ps, cos_sb[:, lsl])
                        rt = tpool.tile([128, TCH], F32, tag="rt")
                        nc.vector.tensor_mul(
                            rt[0:64], ps[64:128], rsin_sb[0:64, lsl]
                        )
                        nc.vector.tensor_mul(
                            rt[64:128], ps[0:64], rsin_sb[64:128, lsl]
                        )
                        nc.vector.tensor_add(qk_sb[bi][pi][:, ft, lsl], ro, rt)

                # k first, q second, v last (wv arrives after wk/wq at start)
                proj_rope(1)
                proj_rope(0)
                for ti in range(TCH // 128):
                    # V token-major: stationary = x slice, moving = Wv
                    ps = ps_a.tile([128, TCH], F32, tag="a", name="psv")
                    for di in range(dt_):
                        nc.tensor.matmul(
                            ps[:, 0:F_LOC],
                            x_sb[:, di, ti * 128 : (ti + 1) * 128],
                            w_sbs[2][:, di, :],
                            start=(di == 0),
                            stop=(di == dt_ - 1),
                        )
                        maybe_pop(F_LOC / 512.0)
                    j0 = c * (TCH // 128) + ti
                    nc.scalar.copy(v_sb[bi][:, j0, :], ps[:, 0:F_LOC])
                chunk_done.add((bi, c))
                maybe_pop(0.0)

            def s2_block(bi, h, qc):
                """PV + denominator + normalize for one (batch, head,
                512-query chunk); E tiles come from the global pipeline."""
                if h == 0 and qc == 0:
                    attn_sb[bi] = attnp.tile(
                        [128, HPC, t], BF16, name=f"an{bi}", tag="an"
                    )
                blk = (bi, h, qc)
                pv = ps_pv.tile([128, QCH], F32, tag="pv")
                for kti in range(kt):
                    while e_cnt.get(blk, 0) < min(kt, kti + 5):
                        if not pop_one(force=True):
                            raise RuntimeError(f"score pipeline stuck at {blk}")
                    if blk not in dn_ps and kti >= 2 and e_cnt[blk] == kt:
                        emit_dn(blk)
                    nc.tensor.matmul(
                        pv,
                        v_sb[bi][:, kti, h * 128 : (h + 1) * 128],
                        e_reg[blk][kti],
                        start=(kti == 0),
                        stop=(kti == kt - 1),
                    )
                    e_reg[blk].pop(kti)
                    state["e_out"] -= 1
                    maybe_pop(1.0)
                e_reg.pop(blk, None)
                # dn_ps[blk] was emitted by the pipeline at this block's
                # 16th pop; reciprocal + normalize fuse into two DVE ops
                rec = s2pool.tile([128, QCH], F32, tag="rec")
                nc.vector.reciprocal_approx_fast(rec, dn_ps.pop(blk))
                nc.vector.tensor_mul(attn_sb[bi][:, h, qc * QCH : (qc + 1) * QCH], pv, rec)

            def s3_block(bi, c4):
                """Out-projection partial for 512 tokens of batch bi,
                drained in four 4-do pieces."""
                off = c4 * TCH
                last = bi == b - 1 and c4 == cpb - 1
                for p4 in range(dt_ // 4):
                    osb = s3pool.tile([128, 4, TCH], FP16, tag="o", name="osb")
                    for dj in range(4):
                        do = p4 * 4 + dj
                        pool_, ptag = (ps_a, "a") if do % 2 == 0 else (ps_pv, "pv")
                        ps = pool_.tile([128, TCH], F32, tag=ptag)
                        for ft in range(HPC):
                            nc.tensor.matmul(
                                ps,
                                wo_sb[:, ft, do * 128 : (do + 1) * 128],
                                attn_sb[bi][:, ft, off : off + TCH],
                                start=(ft == 0),
                                stop=(ft == HPC - 1),
                            )
                            maybe_pop(1.0)
                        # copies 3:1 vector:scalar - the scalar engine is
                        # EXP-saturated during s2/s3 riffles
                        if do % 4 == 0:
                            nc.scalar.copy(osb[:, dj, :], ps)
                        else:
                            nc.vector.tensor_copy(osb[:, dj, :], ps)
                    gsl = slice(bi * t + off, bi * t + off + TCH)
                    dst = outP[:, gsl].rearrange("(do p) tt -> p do tt", p=128)
                    if last:
                        ring = (nc.sync, nc.scalar, nc.gpsimd, nc.sync)[p4]
                    else:
                        ring = nc.sync
                    ring.dma_start(
                        out=dst[:, p4 * 4 : (p4 + 1) * 4, :], in_=osb
                    )

            # ================= riffled emission ========================
            s1_load(0, 1)
            x_tiles[(0, 0)] = x_first
            for c in range(cpb):
                s1_chunk(0, c)
                if c + 2 < cpb:
                    s1_load(0, c + 2)
            for bi in range(b):
                plan = [
                    ("s1l", bi + 1, 0),
                    ("s2", bi, 0, 0), ("s1l", bi + 1, 1), ("s2", bi, 0, 1),
                    ("s1", bi + 1, 0),
                    ("s2", bi, 0, 2), ("s3", bi - 1, 0), ("s2", bi, 0, 3),
                    ("s1", bi + 1, 1), ("s1l", bi + 1, 2),
                    ("s2", bi, 1, 0), ("s3", bi - 1, 1), ("s2", bi, 1, 1),
                    ("s1", bi + 1, 2), ("s1l", bi + 1, 3),
                    ("s2", bi, 1, 2), ("s3", bi - 1, 2),
                    ("s2", bi, 1, 3), ("s1", bi + 1, 3),
                    ("s3", bi - 1, 3),
                ]
                for unit in plan:
                    kind = unit[0]
                    if kind == "s1l" and unit[1] < b:
                        s1_load(unit[1], unit[2])
                    elif kind == "s1" and unit[1] < b:
                        s1_chunk(unit[1], unit[2])
                    elif kind == "s2":
                        s2_block(unit[1], unit[2], unit[3])
                    elif kind == "s3" and unit[1] >= 0:
                        s3_block(unit[1], unit[2])
            for c4 in range(cpb):
                s3_block(b - 1, c4)

    nc.finalize()
    return nc


_module_cache = {}


def _get_module(b, t, d_model, n_cores):
    key = (b, t, d_model, n_cores)
    if key not in _module_cache:
        _module_cache[key] = build_module(b, t, d_model, n_cores)
    return _module_cache[key]


def _host_tables(t):
    half = HEAD_DIM // 2
    theta = 1.0 / (
        np.float32(ROPE_BASE)
        ** (np.arange(half, dtype=np.float32) / np.float32(half))
    )
    freqs = np.arange(t, dtype=np.float32)[:, None] * theta[None, :]
    emb = np.concatenate([freqs, freqs], axis=-1)  # (t, 128)
    bf16 = ml_dtypes.bfloat16
    cosT = np.ascontiguousarray(np.cos(emb).T.astype(bf16))
    sinT = np.sin(emb).T.astype(np.float32)
    rsinT = sinT.copy()
    rsinT[:half] = -sinT[:half]
    rsinT = np.ascontiguousarray(rsinT.astype(bf16))
    return cosT, rsinT


def _run(x, Wq, Wk, Wv, Wo, trace=False):
    b_, t_, d_ = x.shape
    n_cores = (d_ // HEAD_DIM) // HPC
    nc = _get_module(b_, t_, d_, n_cores)

    bf16 = ml_dtypes.bfloat16
    xT = np.ascontiguousarray(x.reshape(b_ * t_, d_).T.astype(bf16))
    cosT, rsinT = _host_tables(t_)

    in_maps = []
    for c in range(n_cores):
        fs = slice(c * F_LOC, (c + 1) * F_LOC)
        in_maps.append(
            {
                "xT": xT,
                "wqT": np.ascontiguousarray(Wq[fs, :].T.astype(bf16)),
                "wkT": np.ascontiguousarray(Wk[fs, :].T.astype(bf16)),
                "wvT": np.ascontiguousarray(Wv[fs, :].T.astype(bf16)),
                "woT": np.ascontiguousarray(Wo[:, fs].T.astype(bf16)),
                "cosT": cosT,
                "rsinT": rsinT,
            }
        )
    res = run_bass_kernel_spmd(
        nc, in_maps, core_ids=list(range(n_cores)), trace=trace
    )
    acc = res.results[0]["outP"].astype(np.float32)
    for c in range(1, n_cores):
        acc += res.results[c]["outP"].astype(np.float32)
    out = np.ascontiguousarray(acc.T).reshape(b_, t_, d_)
    return out, res


def kernel(x, Wq, Wk, Wv, Wo):
    x = np.asarray(x, dtype=np.float32)
    Wq = np.asarray(Wq, dtype=np.float32)
    Wk = np.asarray(Wk, dtype=np.float32)
    Wv = np.asarray(Wv, dtype=np.float32)
    Wo = np.asarray(Wo, dtype=np.float32)
    out, _ = _run(x, Wq, Wk, Wv, Wo, trace=False)
    return out


# revision 18
# speedup vs baseline: 1.0473x; 1.0473x over previous
"""Trainium2 Bass kernel: 16-head RoPE attention block (B=4, T=2048, D=2048).

Sharding: tensor-parallel over heads. Each of the 8 cores owns 2 heads
(a 256-wide slice of the q/k/v projection output features) and computes a
full-width partial of the output projection; the host sums the 8 fp16
partials (the "all-reduce").

v3 structure (vs the riffled v2):
  - cross-block score pipeline: all score matmuls + EXPs flow through one
    global rate-limited queue, popped between every ~3 PE matmuls anywhere
    in the schedule. The scalar engine's EXP backlog builds during the
    projection/out-proj phases, so PV matmuls consume pre-built E tiles
    and never starve on the activation engine.
  - softmax denominators from a depth-3 bf16 add tree (8 pairs -> 4 quads
    -> 2 octs on the DVE) + two ones-matmuls per block (was 5).
  - V produced token-major directly by swapping matmul operands
    (stationary = x slice, moving = Wv) - no XBAR DMA transpose.
  - startup: ones via memset (no DRAM const), x chunk 0 / wk / wv split
    into pieces across the 4 DMA rings, ~shorter PE warm-up.
  - tail: the final out-projection block's DMA drains in 4 pieces across
    all 4 rings.
  - everything flows in bf16 (weights, x, q/k/v, exp(S), attention, Wo);
    PSUM accumulation stays fp32; outputs written as fp16 partials.
"""

import math

import numpy as np
import ml_dtypes

import concourse.bacc as bacc
import concourse.bass as bass
import concourse.mybir as mybir
import concourse.tile as tile
from concourse.bass_utils import run_bass_kernel_spmd

F32 = mybir.dt.float32
BF16 = mybir.dt.bfloat16
FP16 = mybir.dt.float16
EXP = mybir.ActivationFunctionType.Exp

# Problem shape (hardcoded; the harness calls kernel() with exactly these).
B = 4
T = 2048
D_MODEL = 2048
HEAD_DIM = 128
N_CORES = 8
ROPE_BASE = 10000.0

HPC = 2                      # heads per core
F_LOC = HPC * HEAD_DIM       # 256 local projection features per core
TCH = 512                    # stage-1 token chunk width
QCH = 512                    # stage-2 query chunk width
SCALE = 1.0 / math.sqrt(HEAD_DIM)

POP_SLOTS = 2.8              # PE 512-col-slots between score pops
E_MAX = 11                   # outstanding (popped, not PV-consumed) E tiles
WARMUP = 150                 # PE warm-up matmuls (p-state ramp)


def build_module(b=B, t=T, d_model=D_MODEL, n_cores=N_CORES):
    """Build the per-core Bass module. All cores run the same program on
    different data (pure SPMD, no collectives)."""
    dt_ = d_model // 128     # 16 contraction tiles
    kt = t // 128            # 16 key tiles per batch
    cpb = t // TCH           # 4 stage-1 chunks per batch
    nqc = t // QCH           # 4 query chunks

    nc = bacc.Bacc(None, target_bir_lowering=False)

    xT = nc.dram_tensor("xT", [d_model, b * t], BF16, kind="ExternalInput")
    wqT = nc.dram_tensor("wqT", [d_model, F_LOC], BF16, kind="ExternalInput")
    wkT = nc.dram_tensor("wkT", [d_model, F_LOC], BF16, kind="ExternalInput")
    wvT = nc.dram_tensor("wvT", [d_model, F_LOC], BF16, kind="ExternalInput")
    woT = nc.dram_tensor("woT", [F_LOC, d_model], BF16, kind="ExternalInput")
    cosT = nc.dram_tensor("cosT", [HEAD_DIM, t], BF16, kind="ExternalInput")
    rsinT = nc.dram_tensor("rsinT", [HEAD_DIM, t], BF16, kind="ExternalInput")
    outP = nc.dram_tensor("outP", [d_model, b * t], FP16, kind="ExternalOutput")

    with tile.TileContext(nc) as tc:
        with (
            tc.tile_pool(name="const", bufs=1) as constp,
            tc.tile_pool(name="wq", bufs=1) as wpool,
            tc.tile_pool(name="x", bufs=2) as xpool,
            tc.tile_pool(name="qkv", bufs=2) as qkvp,
            tc.tile_pool(name="t1", bufs=2) as tpool,
            tc.tile_pool(name="e", bufs=13) as epool,
            tc.tile_pool(name="tr", bufs=10) as trpool,
            tc.tile_pool(name="s2", bufs=2) as s2pool,
            tc.tile_pool(name="attn", bufs=2) as attnp,
            tc.tile_pool(name="s3o", bufs=6) as s3pool,
            tc.tile_pool(name="ps_a", bufs=2, space="PSUM") as ps_a,
            tc.tile_pool(name="ps_sc", bufs=2, space="PSUM") as ps_sc,
            tc.tile_pool(name="ps_pv", bufs=2, space="PSUM") as ps_pv,
            tc.tile_pool(name="ps_dn", bufs=2, space="PSUM") as ps_dn,
        ):
            # ---- constants: ones from memset (no DRAM), so the PE
            # warm-up starts immediately ----
            ones_sb = constp.tile([128, 128], BF16)
            nc.vector.memset(ones_sb, 1.0)

            # PE warm-up: ramp the p-state while the weight/x DMAs land
            warm_ps = ps_dn.tile([128, QCH], F32, tag="dn")
            for wu in range(WARMUP):
                nc.tensor.matmul(
                    warm_ps[:, 0:128], ones_sb, ones_sb, start=True, stop=True
                )

            # ---- initial loads: wk leads sync+scalar, x chunk 0 3-way
            # across the rings (sync/scalar HWDGE + gpsimd SWDGE), wq
            # behind x, wv on gpsimd. Steady-state x rides gpsimd only so
            # the scalar ring never blocks EXP dispatch. ----
            x_first = xpool.tile([128, dt_, TCH], BF16, name="x0_0", tag="x")
            x0src = xT[:, 0:TCH].rearrange("(dt p) tt -> p dt tt", p=128)

            w_sbs = []
            for wten, wname in ((wqT, "wq"), (wkT, "wk"), (wvT, "wv")):
                wsb = wpool.tile([128, dt_, F_LOC], BF16, name=wname, tag=wname)
                w_sbs.append(wsb)
            wsrc = [
                w.rearrange("(dt p) f -> p dt f", p=128)
                for w in (wqT[:, :], wkT[:, :], wvT[:, :])
            ]
            # wk halves first (needed first)
            nc.sync.dma_start(out=w_sbs[1][:, 0:8, :], in_=wsrc[1][:, 0:8, :])
            nc.scalar.dma_start(out=w_sbs[1][:, 8:16, :], in_=wsrc[1][:, 8:16, :])
            # x chunk 0 split 3 ways
            nc.sync.dma_start(out=x_first[:, 0:5, :], in_=x0src[:, 0:5, :])
            nc.scalar.dma_start(out=x_first[:, 5:10, :], in_=x0src[:, 5:10, :])
            nc.gpsimd.dma_start(out=x_first[:, 10:16, :], in_=x0src[:, 10:16, :])
            # wq halves (q projection is second), wv whole on gpsimd
            nc.sync.dma_start(out=w_sbs[0][:, 0:8, :], in_=wsrc[0][:, 0:8, :])
            nc.scalar.dma_start(out=w_sbs[0][:, 8:16, :], in_=wsrc[0][:, 8:16, :])
            nc.gpsimd.dma_start(out=w_sbs[2], in_=wsrc[2])
            # rope tables (bf16) on scalar; wo trails on gpsimd (issued
            # after the x(0,2) load below)
            cos_sb = constp.tile([128, t], BF16)
            nc.scalar.dma_start(out=cos_sb, in_=cosT[:, :])
            rsin_sb = constp.tile([128, t], BF16)
            nc.scalar.dma_start(out=rsin_sb, in_=rsinT[:, :])
            wo_sb = wpool.tile([128, HPC, d_model], BF16, tag="wo")

            # per-batch double-buffered SBUF state, created lazily
            qk_sb = {}       # bi -> (q_sb, k_sb)  [128, HPC, t] bf16
            v_sb = {}        # bi -> [128, kt, HPC, 128] bf16
            attn_sb = {}     # bi -> [128, HPC, t] bf16

            # ============== global score pipeline ======================
            # Every (bi, h, qc, kti) score matmul + EXP flows through this
            # queue in block order. Pops are rate-limited to one per
            # ~POP_SLOTS 512-col PE slots (the EXP drain rate) and gated
            # on (a) the producing s1 chunks being emitted and (b) at most
            # E_MAX un-consumed E tiles outstanding.
            squeue = []
            for bi_ in range(b):
                for h_ in range(HPC):
                    for qc_ in range(nqc):
                        for kti_ in range(kt):
                            squeue.append((bi_, h_, qc_, kti_))
            state = {"head": 0, "slots": 0.0, "e_out": 0}
            chunk_done = set()
            e_reg = {}       # (bi,h,qc) -> {kti: e_tile}
            e_cnt = {}       # (bi,h,qc) -> popped count
            tree = {}        # (bi,h,qc) -> dict(pr=[], qd=[], oct=[])
            dn_ps = {}       # (bi,h,qc) -> dn psum tile

            def tree_update(blk, kti):
                st = tree.setdefault(blk, {"pr": [], "qd": []})
                reg = e_reg[blk]
                if kti % 2 == 1:
                    pr = trpool.tile([128, QCH], BF16, tag="tr", name="pr")
                    nc.vector.tensor_add(pr, reg[kti - 1], reg[kti])
                    st["pr"].append(pr)
                    if len(st["pr"]) % 2 == 0:
                        qd = trpool.tile([128, QCH], BF16, tag="tr", name="qd")
                        nc.vector.tensor_add(qd, st["pr"][-2], st["pr"][-1])
                        st["qd"].append(qd)

            def emit_dn(blk):
                """Four ones-matmuls over the quad tiles; deferred until the
                PE is safely past the tree's cross-engine latency."""
                st = tree.pop(blk)
                dnp = ps_dn.tile([128, QCH], F32, tag="dn")
                for qi, qd in enumerate(st["qd"]):
                    nc.tensor.matmul(
                        dnp, ones_sb, qd, start=(qi == 0), stop=(qi == 3)
                    )
                dn_ps[blk] = dnp

            def poppable():
                if state["head"] >= len(squeue):
                    return False
                bi_, h_, qc_, kti_ = squeue[state["head"]]
                if (bi_, max(qc_, kti_ // 4)) not in chunk_done:
                    return False
                return state["e_out"] < E_MAX

            def pop_one(force=False):
                if state["head"] >= len(squeue):
                    return False
                bi_, h_, qc_, kti_ = squeue[state["head"]]
                if (bi_, max(qc_, kti_ // 4)) not in chunk_done:
                    return False
                if not force and state["e_out"] >= E_MAX:
                    return False
                state["head"] += 1
                blk = (bi_, h_, qc_)
                q_t, k_t = qk_sb[bi_]
                sps = ps_sc.tile([128, QCH], F32, tag="sc")
                nc.tensor.matmul(
                    sps,
                    k_t[:, h_, kti_ * 128 : (kti_ + 1) * 128],
                    q_t[:, h_, qc_ * QCH : (qc_ + 1) * QCH],
                    start=True,
                    stop=True,
                )
                e_sb = epool.tile([128, QCH], BF16, tag="E", name="e")
                nc.scalar.activation(e_sb, sps, EXP, scale=SCALE)
                e_reg.setdefault(blk, {})[kti_] = e_sb
                e_cnt[blk] = e_cnt.get(blk, 0) + 1
                state["e_out"] += 1
                tree_update(blk, kti_)
                return True

            def maybe_pop(w):
                state["slots"] += w
                while state["slots"] >= POP_SLOTS and pop_one():
                    state["slots"] -= POP_SLOTS
                # when gated or drained, don't bank more than one pop
                state["slots"] = min(state["slots"], POP_SLOTS)

            # ================= emission units =========================
            x_tiles = {}

            def s1_load(bi, c):
                """Issue the x-chunk DMA (placed ahead of its compute).
                Steady state rides the gpsimd SWDGE ring only; chunk (0,1)
                splits sync+gpsimd to beat the startup crunch."""
                off = c * TCH
                tsl = slice(bi * t + off, bi * t + off + TCH)
                x_sb = xpool.tile([128, dt_, TCH], BF16, name=f"x{bi}_{c}", tag="x")
                xsrc = xT[:, tsl].rearrange("(dt p) tt -> p dt tt", p=128)
                if (bi, c) == (0, 1):
                    nc.sync.dma_start(out=x_sb[:, 0:8, :], in_=xsrc[:, 0:8, :])
                    nc.gpsimd.dma_start(out=x_sb[:, 8:16, :], in_=xsrc[:, 8:16, :])
                else:
                    nc.gpsimd.dma_start(out=x_sb, in_=xsrc)
                if (bi, c) == (0, 2):
                    # wo behind the first steady x chunk on gpsimd
                    nc.gpsimd.dma_start(
                        out=wo_sb,
                        in_=woT[:, :].rearrange("(ft p) d -> p ft d", p=128),
                    )
                x_tiles[(bi, c)] = x_sb

            def s1_chunk(bi, c):
                """Projections + rope + token-major V for 512 tokens."""
                if c == 0:
                    qk_sb[bi] = (
                        qkvp.tile([128, HPC, t], BF16, name=f"q{bi}", tag="q"),
                        qkvp.tile([128, HPC, t], BF16, name=f"k{bi}", tag="k"),
                    )
                    v_sb[bi] = qkvp.tile(
                        [128, kt, F_LOC], BF16, name=f"v{bi}", tag="v"
                    )
                off = c * TCH
                lsl = slice(off, off + TCH)
                x_sb = x_tiles.pop((bi, c))

                def proj_rope(pi):
                    for ft in range(HPC):
                        fsl = slice(ft * 128, (ft + 1) * 128)
                        ps = ps_a.tile([128, TCH], F32, tag="a", name="psqk")
                        for di in range(dt_):
                            nc.tensor.matmul(
                                ps,
                                w_sbs[pi][:, di, fsl],
                                x_sb[:, di, :],
                                start=(di == 0),
                                stop=(di == dt_ - 1),
                            )
                            maybe_pop(1.0)
                        # rope: out = in*cos + rot_half(in)*sin
                        ro = tpool.tile([128, TCH], F32, tag="ro")
                        nc.vector.tensor_mul(ro, ps, cos_sb[:, lsl])
                        rt = tpool.tile([128, TCH], F32, tag="rt")
                        nc.vector.tensor_mul(
                            rt[0:64], ps[64:128], rsin_sb[0:64, lsl]
                        )
                        nc.vector.tensor_mul(
                            rt[64:128], ps[0:64], rsin_sb[64:128, lsl]
                        )
                        nc.vector.tensor_add(qk_sb[bi][pi][:, ft, lsl], ro, rt)

                # k first, q second, v last (wv arrives after wk/wq at start)
                proj_rope(1)
                proj_rope(0)
                for ti in range(TCH // 128):
                    # V token-major: stationary = x slice, moving = Wv
                    ps = ps_a.tile([128, TCH], F32, tag="a", name="psv")
                    for di in range(dt_):
                        nc.tensor.matmul(
                            ps[:, 0:F_LOC],
                            x_sb[:, di, ti * 128 : (ti + 1) * 128],
                            w_sbs[2][:, di, :],
                            start=(di == 0),
                            stop=(di == dt_ - 1),
                        )
                        maybe_pop(F_LOC / 512.0)
                    j0 = c * (TCH // 128) + ti
                    nc.scalar.copy(v_sb[bi][:, j0, :], ps[:, 0:F_LOC])
                chunk_done.add((bi, c))
                maybe_pop(0.0)

            def s2_block(bi, h, qc):
                """PV + denominator + normalize for one (batch, head,
                512-query chunk); E tiles come from the global pipeline."""
                if h == 0 and qc == 0:
                    attn_sb[bi] = attnp.tile(
                        [128, HPC, t], BF16, name=f"an{bi}", tag="an"
                    )
                blk = (bi, h, qc)
                pv = ps_pv.tile([128, QCH], F32, tag="pv")
                for kti in range(kt):
                    while e_cnt.get(blk, 0) < min(kt, kti + 5):
                        if not pop_one(force=True):
                            raise RuntimeError(f"score pipeline stuck at {blk}")
                    if blk not in dn_ps and kti >= 2 and e_cnt[blk] == kt:
                        emit_dn(blk)
                    nc.tensor.matmul(
                        pv,
                        v_sb[bi][:, kti, h * 128 : (h + 1) * 128],
                        e_reg[blk][kti],
                        start=(kti == 0),
                        stop=(kti == kt - 1),
                    )
                    e_reg[blk].pop(kti)
                    state["e_out"] -= 1
                    maybe_pop(1.0)
                e_reg.pop(blk, None)
                # dn_ps[blk] was emitted by the pipeline at this block's
                # 16th pop; reciprocal + normalize fuse into two DVE ops
                rec = s2pool.tile([128, QCH], F32, tag="rec")
                nc.vector.reciprocal_approx_fast(rec, dn_ps.pop(blk))
                nc.vector.tensor_mul(attn_sb[bi][:, h, qc * QCH : (qc + 1) * QCH], pv, rec)

            def s3_quarter(bi, c4, p4):
                """Out-projection partial for 4 of 16 output row-blocks of
                one 512-token chunk of batch bi; riffled finely so the
                psum->fp16 copies spread across the whole schedule."""
                off = c4 * TCH
                last = bi == b - 1 and c4 == cpb - 1
                osb = s3pool.tile([128, 4, TCH], FP16, tag="o", name="osb")
                for dj in range(4):
                    do = p4 * 4 + dj
                    pool_, ptag = (ps_a, "a") if do % 2 == 0 else (ps_pv, "pv")
                    ps = pool_.tile([128, TCH], F32, tag=ptag)
                    for ft in range(HPC):
                        nc.tensor.matmul(
                            ps,
                            wo_sb[:, ft, do * 128 : (do + 1) * 128],
                            attn_sb[bi][:, ft, off : off + TCH],
                            start=(ft == 0),
                            stop=(ft == HPC - 1),
                        )
                        maybe_pop(1.0)
                    if do % 2 == 0:
                        nc.scalar.copy(osb[:, dj, :], ps)
                    else:
                        nc.vector.tensor_copy(osb[:, dj, :], ps)
                gsl = slice(bi * t + off, bi * t + off + TCH)
                dst = outP[:, gsl].rearrange("(do p) tt -> p do tt", p=128)
                if last:
                    ring = (nc.sync, nc.scalar, nc.gpsimd, nc.sync)[p4]
                else:
                    ring = nc.sync
                ring.dma_start(out=dst[:, p4 * 4 : (p4 + 1) * 4, :], in_=osb)

            # ================= riffled emission ========================
            s1_load(0, 1)
            x_tiles[(0, 0)] = x_first
            for c in range(cpb):
                s1_chunk(0, c)
                if c + 2 < cpb:
                    s1_load(0, c + 2)
            for bi in range(b):
                # s3 quarter-blocks of batch bi-1 riffle between the s2
                # blocks of bi and s1 chunks of bi+1 (q = running index
                # 0..15 over (c4, p4) pairs)
                plan = [
                    ("s1l", bi + 1, 0),
                    ("s2", bi, 0, 0), ("s3q", 0), ("s2", bi, 0, 1),
                    ("s3q", 1), ("s3q", 2),
                    ("s1l", bi + 1, 1), ("s1", bi + 1, 0),
                    ("s3q", 3), ("s2", bi, 0, 2), ("s3q", 4),
                    ("s2", bi, 0, 3), ("s3q", 5), ("s3q", 6),
                    ("s1l", bi + 1, 2), ("s1", bi + 1, 1),
                    ("s3q", 7), ("s2", bi, 1, 0), ("s3q", 8),
                    ("s2", bi, 1, 1), ("s3q", 9), ("s3q", 10),
                    ("s1l", bi + 1, 3), ("s1", bi + 1, 2),
                    ("s3q", 11), ("s2", bi, 1, 2), ("s3q", 12),
                    ("s2", bi, 1, 3), ("s3q", 13), ("s3q", 14),
                    ("s1", bi + 1, 3),
                    ("s3q", 15),
                ]
                for unit in plan:
                    kind = unit[0]
                    if kind == "s1l" and unit[1] < b:
                        s1_load(unit[1], unit[2])
                    elif kind == "s1" and unit[1] < b:
                        s1_chunk(unit[1], unit[2])
                    elif kind == "s2":
                        s2_block(unit[1], unit[2], unit[3])
                    elif kind == "s3q" and bi > 0:
                        s3_quarter(bi - 1, unit[1] // 4, unit[1] % 4)
            for q_ in range(cpb * 4):
                s3_quarter(b - 1, q_ // 4, q_ % 4)

    nc.finalize()
    return nc


_module_cache = {}


def _get_module(b, t, d_model, n_cores):
    key = (b, t, d_model, n_cores)
    if key not in _module_cache:
        _module_cache[key] = build_module(b, t, d_model, n_cores)
    return _module_cache[key]


def _host_tables(t):
    half = HEAD_DIM // 2
    theta = 1.0 / (
        np.float32(ROPE_BASE)
        ** (np.arange(half, dtype=np.float32) / np.float32(half))
    )
    freqs = np.arange(t, dtype=np.float32)[:, None] * theta[None, :]
    emb = np.concatenate([freqs, freqs], axis=-1)  # (t, 128)
    bf16 = ml_dtypes.bfloat16
    cosT = np.ascontiguousarray(np.cos(emb).T.astype(bf16))
    sinT = np.sin(emb).T.astype(np.float32)
    rsinT = sinT.copy()
    rsinT[:half] = -sinT[:half]
    rsinT = np.ascontiguousarray(rsinT.astype(bf16))
    return cosT, rsinT


def _run(x, Wq, Wk, Wv, Wo, trace=False):
    b_, t_, d_ = x.shape
    n_cores = (d_ // HEAD_DIM) // HPC
    nc = _get_module(b_, t_, d_, n_cores)

    bf16 = ml_dtypes.bfloat16
    xT = np.ascontiguousarray(x.reshape(b_ * t_, d_).T.astype(bf16))
    cosT, rsinT = _host_tables(t_)

    in_maps = []
    for c in range(n_cores):
        fs = slice(c * F_LOC, (c + 1) * F_LOC)
        in_maps.append(
            {
                "xT": xT,
                "wqT": np.ascontiguousarray(Wq[fs, :].T.astype(bf16)),
                "wkT": np.ascontiguousarray(Wk[fs, :].T.astype(bf16)),
                "wvT": np.ascontiguousarray(Wv[fs, :].T.astype(bf16)),
                "woT": np.ascontiguousarray(Wo[:, fs].T.astype(bf16)),
                "cosT": cosT,
                "rsinT": rsinT,
            }
        )
    res = run_bass_kernel_spmd(
        nc, in_maps, core_ids=list(range(n_cores)), trace=trace
    )
    acc = res.results[0]["outP"].astype(np.float32)
    for c in range(1, n_cores):
        acc += res.results[c]["outP"].astype(np.float32)
    out = np.ascontiguousarray(acc.T).reshape(b_, t_, d_)
    return out, res


def kernel(x, Wq, Wk, Wv, Wo):
    x = np.asarray(x, dtype=np.float32)
    Wq = np.asarray(Wq, dtype=np.float32)
    Wk = np.asarray(Wk, dtype=np.float32)
    Wv = np.asarray(Wv, dtype=np.float32)
    Wo = np.asarray(Wo, dtype=np.float32)
    out, _ = _run(x, Wq, Wk, Wv, Wo, trace=False)
    return out
